# revision 1
# baseline (speedup 1.0000x reference)
"""Chebyshev approximation kernel for Trainium2 (8 NeuronCores, SPMD data-parallel).

Math: reference computes
    y_at_nodes = (1-t) * y[:, idx] + t * y[:, idx+1]      # [n_obs, deg]
    out        = (y_at_nodes @ basis).reshape(-1)         # [n_obs*deg]
Both steps are linear in y, so we fold them into a single matrix on host:
    C[k, d] = sum_j W[k, j] * basis[j, d],   W = interp weights (2 nnz/col)
    out     = y @ C          # [n_obs, 2049] @ [2049, 1024]
The device kernel is one GEMM per 128-row block: PE-transpose the y block
(grid axis onto partitions, float32r transpose-mode matmuls into PSUM, drained
by wide DVE/ACT copies), then 16 accumulating float32r matmuls (k-tiles of
128) per 512-wide output half; grid column 2048's rank-1 contribution is
folded on DVE during the output copy. float32r = fp32 storage with FP22
multiplies at full PE rate (1 cycle/row for N>=256), fp32 accumulation.

Sharding: y rows split 8192/core across 8 cores; C replicated.
"""

import os
import numpy as np

DEG = 1024
N_OBS = 65536
M_P1 = 2049
N_CORES = 8
ROWS_PER_CORE = N_OBS // N_CORES  # 8192
KT = 17                           # contraction tiles of 128 (2049 -> 2176 padded)
KP = KT * 128                     # 2176
RB = 128                          # rows per block

_COMPILED = {}
LAST_RESULTS = None


def _cheb_c_matrix(x: np.ndarray) -> np.ndarray:
    """C [KP, DEG] float32 with zero pad rows >= 2049; out = y @ C[:M_P1]."""
    x = np.asarray(x, dtype=np.float32)
    k = np.arange(DEG, dtype=np.float32)
    # float32 node computation, mimicking the jax reference
    ang = (np.float32(np.pi) * (k + np.float32(0.5))) / np.float32(DEG)
    nodes = np.sort(np.cos(ang.astype(np.float32)).astype(np.float32))
    norm = ((np.float32(2.0) - (k == 0).astype(np.float32)) / np.float32(DEG)).astype(
        np.float64
    )
    # basis[j, d] = norm_d * cos(d * arccos(node_j)); f64 from f32 nodes
    theta = np.arccos(nodes.astype(np.float64))
    basis = norm[None, :] * np.cos(k.astype(np.float64)[None, :] * theta[:, None])
    idx = np.clip(np.searchsorted(x, nodes, side="right") - 1, 0, M_P1 - 2)
    a = x[idx]
    b = x[idx + 1]
    t = ((nodes - a) / (b - a)).astype(np.float64)
    C = np.zeros((KP, DEG), dtype=np.float64)
    np.add.at(C, idx, (1.0 - t)[:, None] * basis)
    np.add.at(C, idx + 1, t[:, None] * basis)
    return np.ascontiguousarray(C.astype(np.float32))


def build_cheb_kernel(tc, y_ap, c_ap, id_ap, o_ap, rows):
    """Emit the per-core program: out[rows, DEG] = y[rows, M_P1] @ C[:M_P1]."""
    import concourse.mybir as mybir

    nc = tc.nc
    f32 = mybir.dt.float32
    f32r = mybir.dt.float32r
    nblocks = rows // RB

    # 16 full k-tiles cover columns 0..2047; column 2048's rank-1 update is
    # folded on DVE during the output copy (out += y[:,2048] * C[2048,:]).
    KTM = 16
    # Stages grouped 4-per-PSUM-bank: a burst of 4 PE transposes shares one
    # PSUM bank, drained by a single wide copy; main matmuls run one group
    # behind so the drain is off their critical path.
    G = 4

    with (
        tc.tile_pool(name="consts", bufs=1) as consts,
        tc.tile_pool(name="ypool", bufs=4) as ypool,
        tc.tile_pool(name="ytpool", bufs=2) as ytpool,
        tc.tile_pool(name="opool", bufs=3) as opool,
        tc.tile_pool(name="pst", bufs=4, space="PSUM") as pstp,
        tc.tile_pool(name="pso", bufs=2, space="PSUM") as psop,
    ):
        ident = consts.tile([128, 128], f32r)
        nc.sync.dma_start(out=ident, in_=id_ap)
        # C resident in SBUF: [partition-within-tile, ktile, d]; chunked DMAs
        # on the scalar HWDGE queue so y loads (sync queue) aren't blocked.
        # Alternate C chunks between the scalar and sync HWDGE queues:
        # serialized on one queue the 16 chunks take ~24us and the first
        # blocks' matmuls stall waiting for late k-tiles. (gpsimd SWDGE is
        # avoided — its ring setup adds ~5us to engine startup.)
        c_sb = consts.tile([128, KTM, DEG], f32r)
        c_r = c_ap.rearrange("(t p) n -> p t n", p=128)
        def load_c(k):
            eng = nc.scalar if k % 2 == 0 else nc.sync
            eng.dma_start(out=c_sb[:, k, :], in_=c_r[:, k, :])
        # C row 2048 replicated across partitions for the DVE rank-1 fold.
        c_rep = consts.tile([128, DEG], f32)
        import concourse.bass as bass

        c_row = c_ap[KTM * 128 : KTM * 128 + 1, :].bitcast(f32)
        c_row_bc = bass.AP(
            tensor=c_row.tensor, offset=c_row.offset, ap=[[0, 128]] + list(c_row.ap[1:])
        )

        ybs, ytbs, pss = {}, {}, {}

        def load_y(b, split=False):
            yb = ypool.tile([128, M_P1], f32r, name="yb", tag="yb")
            rows = y_ap[b * RB : (b + 1) * RB, :]
            if split:
                # halves so block 0's first transposes start sooner
                nc.sync.dma_start(out=yb[:, 0:1024], in_=rows[:, 0:1024])
                nc.sync.dma_start(out=yb[:, 1024:M_P1], in_=rows[:, 1024:M_P1])
            else:
                nc.sync.dma_start(out=yb, in_=rows)
            ybs[b] = yb

        def emit_t_group(b, g):
            if g == 0:
                ytbs[b] = ytpool.tile([128, KTM, 128], f32r, name="ytb", tag="ytb")
            pst = pstp.tile([128, G, 128], f32r, name="pst", tag="pst")
            for j in range(G):
                k = g * G + j
                nc.tensor.transpose(
                    pst[:, j, :], ybs[b][:, k * 128 : (k + 1) * 128], ident
                )
            dst = ytbs[b][:, g * G : (g + 1) * G, :]
            if g % 2 == 0:
                nc.vector.tensor_copy(dst, pst)
            else:
                nc.scalar.copy(dst, pst)

        def emit_m_group(b, g):
            if g == 0:
                pss[b] = psop.tile([128, DEG], f32, name="ps", tag="ps")
            ps = pss[b]
            for j in range(G):
                k = g * G + j
                for nh in range(2):
                    nc.tensor.matmul(
                        ps[:, nh * 512 : (nh + 1) * 512],
                        ytbs[b][:, k, :],
                        c_sb[:, k, nh * 512 : (nh + 1) * 512],
                        start=(k == 0),
                        stop=(k == KTM - 1),
                    )
            if g == KTM // G - 1:
                tmp = opool.tile([128, DEG], f32, name="tmp", tag="tmp")
                nc.vector.tensor_scalar_mul(
                    tmp, c_rep, ybs[b][:, 2048:2049].bitcast(f32)
                )
                osb = opool.tile([128, DEG], f32, name="osb", tag="osb")
                nc.vector.tensor_add(osb, ps, tmp)
                nc.scalar.dma_start(out=o_ap[b * RB : (b + 1) * RB, :], in_=osb)
                del ybs[b], ytbs[b], pss[b]

        groups = [(b, g) for b in range(nblocks) for g in range(KTM // G)]
        load_y(0, split=True)
        for k in range(KTM):
            load_c(k)
        nc.scalar.dma_start(out=c_rep, in_=c_row_bc)
        for i in range(len(groups) + 1):
            if i < len(groups):
                b, g = groups[i]
                if g == 0 and b + 1 < nblocks:
                    load_y(b + 1)
                emit_t_group(b, g)
            if i >= 1:
                emit_m_group(*groups[i - 1])


def _build_nc(rows):
    import concourse.mybir as mybir
    import concourse.tile as tile
    from concourse import bacc

    f32 = mybir.dt.float32
    f32r = mybir.dt.float32r
    nc = bacc.Bacc(
        "TRN2",
        target_bir_lowering=False,
        debug=False,
        enable_asserts=False,
        num_devices=N_CORES,
    )
    y_ap = nc.dram_tensor("y", [rows, M_P1], f32r, kind="ExternalInput").ap()
    c_ap = nc.dram_tensor("c", [KP, DEG], f32r, kind="ExternalInput").ap()
    id_ap = nc.dram_tensor("ident", [128, 128], f32r, kind="ExternalInput").ap()
    o_ap = nc.dram_tensor("o", [rows, DEG], f32, kind="ExternalOutput").ap()
    with tile.TileContext(nc) as tc:
        build_cheb_kernel(tc, y_ap, c_ap, id_ap, o_ap, rows)
    nc.compile()
    return nc


def _get_compiled(rows=ROWS_PER_CORE):
    if rows not in _COMPILED:
        _COMPILED[rows] = _build_nc(rows)
    return _COMPILED[rows]


def kernel(x: np.ndarray, y: np.ndarray) -> np.ndarray:
    global LAST_RESULTS
    from concourse import bass_utils

    x = np.asarray(x, dtype=np.float32)
    y = np.ascontiguousarray(np.asarray(y, dtype=np.float32))
    assert y.shape == (N_OBS, M_P1), y.shape
    C = _cheb_c_matrix(x)

    nc = _get_compiled()
    ident = np.ascontiguousarray(np.eye(128, dtype=np.float32))
    in_maps = [
        {"y": y[i * ROWS_PER_CORE : (i + 1) * ROWS_PER_CORE], "c": C, "ident": ident}
        for i in range(N_CORES)
    ]
    trace = bool(int(os.environ.get("CHEB_TRACE", "0")))
    res = bass_utils.run_bass_kernel_spmd(
        nc, in_maps, core_ids=list(range(N_CORES)), trace=trace
    )
    LAST_RESULTS = res
    out = np.concatenate([res.results[i]["o"] for i in range(N_CORES)], axis=0)
    return out.reshape(-1)



# revision 9
# speedup vs baseline: 2.1272x; 2.1272x over previous
"""Chebyshev approximation kernel for Trainium2 (8 NeuronCores, SPMD data-parallel).

Math: reference computes
    q   = (1-t) * y[:, idx] + t * y[:, idx+1]     # [n_obs, deg]  (interp at nodes)
    out = (q @ basis).reshape(-1)                 # basis = DCT-II-like matrix

Factorization used here (device does ~4x less PE work than the fused GEMM):
  1. q' = y @ W'            W' = interp matrix with node columns permuted
                            [nodes 0..511, nodes 1023..512]; banded, 2 nnz/col.
  2. DCT-II radix split: u = q'[:, :512] + q'[:, 512:], v = q' first - second
     out[:, 2i]   = (u @ De)[:, i]
     out[:, 2i+1] = (v @ Do)[:, i]     De/Do [512, 512] dense (host-built, f64)
  So the big GEMM has K=512 twice (vs K=2048 once), and step 1 is banded.

Device schedule per 256-row superblock, all matmul dtypes bf16 (1 cyc/row at
any moving size; fp32 PSUM accumulate; rel err ~3.5e-3 vs 2e-2 gate):
  - 34 PE transposes (y tiles -> yT), drained to SBUF by DVE/ACT alternately.
  - step A: ~22 node-stationary banded matmuls (lhsT = W' 128x128 tiles,
    moving = yT k-tile [128, 256]); psum bank b holds node-tiles (b, b+4) --
    the butterfly partners -- in its two halves.
  - butterfly: DVE tensor_tensor add/sub of the two psum halves -> u,v in
    SBUF bf16 (drain + butterfly + cast fused).
  - step B: per 128-row block, 2 accumulation groups x 4 matmuls (K=512,
    N=512) from u/v slices vs De/Do; drain interleaves even/odd coeffs.
  B' of superblock s is emitted during s+1's transposes to cover the
  butterfly latency. Output returned bf16, upcast on host.

Sharding: y rows split 8192/core across 8 cores; constants replicated.
"""

import os
import numpy as np
import ml_dtypes

DEG = 1024
N_OBS = 65536
M_P1 = 2049
N_CORES = 8
ROWS_PER_CORE = N_OBS // N_CORES  # 8192
KT = 17                           # grid k-tiles: 16 full + overlap tile (col 2048)
RB = 128                          # rows per block
SBROWS = 256                      # rows per superblock

_COMPILED = {}
LAST_RESULTS = None


def _build_mats(x: np.ndarray):
    """Host prep: banded interp weight tiles (permuted node order) + DCT-split
    matrices, all f64 -> bf16. Returns (wt [128,P,128], dmat [128,8,512],
    pairs [(k,g)...] sorted)."""
    x = np.asarray(x, dtype=np.float32)
    k = np.arange(DEG, dtype=np.float32)
    ang = (np.float32(np.pi) * (k + np.float32(0.5))) / np.float32(DEG)
    nodes = np.sort(np.cos(ang.astype(np.float32)).astype(np.float32))
    norm = (np.float32(2.0) - (k == 0).astype(np.float32)).astype(np.float64) / float(
        DEG
    )
    idx = np.clip(np.searchsorted(x, nodes, side="right") - 1, 0, M_P1 - 2)
    a = x[idx]
    b = x[idx + 1]
    t = ((nodes - a) / (b - a)).astype(np.float64)

    # permuted node-column order: [0..511, 1023..512]
    perm = np.concatenate([np.arange(512), np.arange(1023, 511, -1)])
    idxp = idx[perm].astype(np.int64)
    tp = t[perm]

    # nnz of W' [2049, 1024]: (row idxp, 1-tp), (row idxp+1, tp)
    rows_ = np.concatenate([idxp, idxp + 1])
    ws = np.concatenate([1.0 - tp, tp])
    cols_ = np.concatenate([np.arange(DEG), np.arange(DEG)])
    # grid row -> (ktile, partition); row 2048 lives in overlap tile 16
    # (tile 16 = grid cols 1921..2048, so col 2048 -> partition 127)
    ktile = np.where(rows_ == 2048, 16, rows_ // 128)
    part = np.where(rows_ == 2048, 127, rows_ % 128)
    g = cols_ // 128

    pairs = sorted(set(zip(ktile.tolist(), g.tolist())))
    pidx = {pg: i for i, pg in enumerate(pairs)}
    wt = np.zeros((128, len(pairs), 128), dtype=np.float64)
    pvec = np.array([pidx[(kk, gg)] for kk, gg in zip(ktile.tolist(), g.tolist())])
    np.add.at(wt, (part, pvec, cols_ % 128), ws)

    n_ = np.arange(512, dtype=np.float64)
    i_ = np.arange(512, dtype=np.float64)
    De = np.cos(np.pi * np.outer(n_ + 0.5, i_) / 512.0) * norm[0::2][None, :]
    Do = -np.cos(np.pi * np.outer(n_ + 0.5, 2.0 * i_ + 1.0) / 1024.0) * norm[1::2][
        None, :
    ]
    dmat = np.zeros((128, 8, 512), dtype=np.float64)
    for gg in range(4):
        dmat[:, gg, :] = De[128 * gg : 128 * (gg + 1), :]
        dmat[:, 4 + gg, :] = Do[128 * gg : 128 * (gg + 1), :]

    bf = ml_dtypes.bfloat16
    return (
        np.ascontiguousarray(wt.astype(np.float32).astype(bf)),
        np.ascontiguousarray(dmat.astype(np.float32).astype(bf)),
        tuple(pairs),
    )


def build_cheb_kernel(tc, y_ap, wt_ap, d_ap, id_ap, o_ap, rows, pairs):
    import concourse.mybir as mybir

    nc = tc.nc
    bf = mybir.dt.bfloat16
    f32 = mybir.dt.float32
    SB = rows // SBROWS
    P = len(pairs)
    pidx = {pg: i for i, pg in enumerate(pairs)}
    pairs_by_k = {}
    for kk, gg in pairs:
        pairs_by_k.setdefault(kk, []).append(gg)
    # per-bank first/last pair (bank b = g % 4, half = g // 4)
    bank_pairs = {b: [] for b in range(4)}
    for kk, gg in pairs:  # sorted by (k, g)
        bank_pairs[gg % 4].append((kk, gg))
    # per node-tile g first/last pair: each half of a bank is its own psum
    # accumulation group (half-0 finishes before half-1 starts, so the bank's
    # zero region only ever has one open group)
    first_of_g = {}
    last_of_g = {}
    for kk, gg in pairs:
        first_of_g.setdefault(gg, (kk, gg))
        last_of_g[gg] = (kk, gg)

    KGROUPS = [(0, 4), (4, 8), (8, 12), (12, 16), (16, 17)]
    add_op = mybir.AluOpType.add
    sub_op = mybir.AluOpType.subtract

    with (
        tc.tile_pool(name="consts", bufs=1) as consts,
        tc.tile_pool(name="ypool", bufs=3) as ypool,
        tc.tile_pool(name="ytpool", bufs=2) as ytpool,
        tc.tile_pool(name="uvpool", bufs=2) as uvpool,
        tc.tile_pool(name="h0pool", bufs=2) as h0pool,
        tc.tile_pool(name="opool", bufs=4) as opool,
        tc.tile_pool(name="pstp", bufs=2, space="PSUM") as pstp,
        tc.tile_pool(name="pnp", bufs=1, space="PSUM") as pnp,
        tc.tile_pool(name="pop", bufs=2, space="PSUM") as pop,
    ):
        ident = consts.tile([128, 128], bf)
        nc.scalar.dma_start(out=ident, in_=id_ap)
        wt_sb = consts.tile([128, P, 128], bf)
        nc.scalar.dma_start(out=wt_sb, in_=wt_ap)
        d_sb = consts.tile([128, 8, 512], bf)
        d_r = d_ap.rearrange("p (a b) -> p a b", a=8)
        for half in range(2):
            eng = nc.sync if half == 0 else nc.scalar
            eng.dma_start(out=d_sb[:, 4 * half : 4 * half + 4, :],
                          in_=d_r[:, 4 * half : 4 * half + 4, :])

        ys = {}

        def load_ys(s):
            ti = ypool.tile([128, 2, M_P1], bf, name="ys", tag="ys")
            for rb in range(2):
                r0 = (s * 2 + rb) * RB
                nc.sync.dma_start(out=ti[:, rb, 0:1024], in_=y_ap[r0 : r0 + RB, 0:1024])
                nc.sync.dma_start(
                    out=ti[:, rb, 1024:M_P1], in_=y_ap[r0 : r0 + RB, 1024:M_P1]
                )
            ys[s] = ti

        def emit_b(uv, s):
            # step B for superblock s: 2 blocks x (even, odd) accumulation groups
            for rb in range(2):
                osb = opool.tile([128, 512, 2], bf, name="osb", tag="osb")
                for parity in range(2):
                    po = pop.tile([128, 512], f32, name="po", tag="po")
                    for gg in range(4):
                        nc.tensor.matmul(
                            po,
                            uv[:, 4 * parity + gg, rb * 128 : (rb + 1) * 128],
                            d_sb[:, 4 * parity + gg, :],
                            start=(gg == 0),
                            stop=(gg == 3),
                        )
                    if parity == 0:
                        nc.vector.tensor_copy(osb[:, :, 0], po)
                    else:
                        nc.scalar.copy(osb[:, :, 1], po)
                r0 = (s * 2 + rb) * RB
                nc.scalar.dma_start(
                    out=o_ap[r0 : r0 + RB, :], in_=osb.rearrange("p a b -> p (a b)")
                )

        load_ys(0)
        prev = None
        for s in range(SB):
            if s + 1 < SB:
                load_ys(s + 1)
            ytb = ytpool.tile([128, KT, SBROWS], bf, name="ytb", tag="ytb")
            pns = [
                pnp.tile([128, 2, SBROWS], f32, name=f"pn{b}", tag=f"pn{b}")
                for b in range(4)
            ]
            uv = uvpool.tile([128, 8, SBROWS], bf, name="uv", tag="uv")
            h0 = h0pool.tile([128, 4, SBROWS], f32, name="h0", tag="h0")
            for gi, (k0, k1) in enumerate(KGROUPS):
                pst = pstp.tile([128, 4, SBROWS], bf, name="pst", tag="pst")
                for kk in range(k0, k1):
                    for rb in range(2):
                        if kk < 16:
                            src = ys[s][:, rb, kk * 128 : (kk + 1) * 128]
                        else:
                            src = ys[s][:, rb, 1921:2049]
                        nc.tensor.transpose(
                            pst[:, kk - k0, rb * 128 : (rb + 1) * 128], src, ident
                        )
                eng = nc.vector if gi % 2 == 0 else nc.scalar
                dst = ytb[:, k0:k1, :]
                srcp = pst[:, 0 : k1 - k0, :]
                if gi % 2 == 0:
                    nc.vector.tensor_copy(dst, srcp)
                else:
                    nc.scalar.copy(dst, srcp)
                # step A matmuls for k-tiles in this group
                for kk in range(k0, k1):
                    for gg in pairs_by_k.get(kk, []):
                        b, h = gg % 4, gg // 4
                        nc.tensor.matmul(
                            pns[b][:, h, :],
                            wt_sb[:, pidx[(kk, gg)], :],
                            ytb[:, kk, :],
                            start=((kk, gg) == first_of_g[gg]),
                            stop=((kk, gg) == last_of_g[gg]),
                        )
                        if (kk, gg) == last_of_g[gg]:
                            if gg < 4:
                                # left half done early: park it in SBUF so the
                                # butterfly has only one PSUM operand
                                nc.scalar.copy(h0[:, gg, :], pns[b][:, 0, :])
                            else:
                                nc.vector.tensor_tensor(
                                    uv[:, b, :], pns[b][:, 1, :], h0[:, b, :], add_op
                                )
                                nc.vector.tensor_tensor(
                                    uv[:, 4 + b, :], h0[:, b, :], pns[b][:, 1, :], sub_op
                                )
            if prev is not None:
                emit_b(*prev)
                del ys[prev[1]]
            prev = (uv, s)
        emit_b(*prev)


def _build_nc(rows, pairs):
    import concourse.mybir as mybir
    import concourse.tile as tile
    from concourse import bacc

    bf = mybir.dt.bfloat16
    P = len(pairs)
    nc = bacc.Bacc(
        "TRN2",
        target_bir_lowering=False,
        debug=False,
        enable_asserts=False,
        num_devices=N_CORES,
    )
    y_ap = nc.dram_tensor("y", [rows, M_P1], bf, kind="ExternalInput").ap()
    wt_ap = nc.dram_tensor("wt", [128, P * 128], bf, kind="ExternalInput").ap()
    d_ap = nc.dram_tensor("dmat", [128, 8 * 512], bf, kind="ExternalInput").ap()
    id_ap = nc.dram_tensor("ident", [128, 128], bf, kind="ExternalInput").ap()
    o_ap = nc.dram_tensor("o", [rows, DEG], bf, kind="ExternalOutput").ap()
    wt_r = wt_ap.rearrange("p (a b) -> p a b", a=P)
    with tile.TileContext(nc) as tc:
        build_cheb_kernel(tc, y_ap, wt_r, d_ap, id_ap, o_ap, rows, pairs)
    nc.compile()
    return nc


def _get_compiled(rows, pairs):
    key = (rows, pairs)
    if key not in _COMPILED:
        _COMPILED[key] = _build_nc(rows, pairs)
    return _COMPILED[key]


def kernel(x: np.ndarray, y: np.ndarray) -> np.ndarray:
    global LAST_RESULTS
    from concourse import bass_utils

    bfd = ml_dtypes.bfloat16
    x = np.asarray(x, dtype=np.float32)
    y = np.asarray(y, dtype=np.float32)
    assert y.shape == (N_OBS, M_P1), y.shape
    yb = np.ascontiguousarray(y.astype(bfd))
    wt, dmat, pairs = _build_mats(x)

    nc = _get_compiled(ROWS_PER_CORE, pairs)
    ident = np.ascontiguousarray(np.eye(128, dtype=np.float32).astype(bfd))
    wt2 = wt.reshape(128, -1)
    d2 = dmat.reshape(128, -1)
    in_maps = [
        {
            "y": yb[i * ROWS_PER_CORE : (i + 1) * ROWS_PER_CORE],
            "wt": wt2,
            "dmat": d2,
            "ident": ident,
        }
        for i in range(N_CORES)
    ]
    trace = bool(int(os.environ.get("CHEB_TRACE", "0")))
    res = bass_utils.run_bass_kernel_spmd(
        nc, in_maps, core_ids=list(range(N_CORES)), trace=trace
    )
    LAST_RESULTS = res
    out = np.concatenate(
        [np.asarray(res.results[i]["o"]) for i in range(N_CORES)], axis=0
    )
    return out.astype(np.float32).reshape(-1)
